# revision 3
# baseline (speedup 1.0000x reference)
"""Causal multi-head self-attention (B=8, S=2048, D=384, H=4, Hd=96) on 8
Trainium2 NeuronCores.

Sharding: data-parallel over batch — each core processes one batch element,
weights replicated. No collectives needed.

Per-core algorithm (flash-style, fully SBUF-resident, no attention matrix in
HBM):
  - host passes x[b] pre-transposed as xT [384, 2048] (layout prep only)
  - QT/KT computed per head in [96, S] layout (d on partitions), cast to bf16
    during the PSUM->SBUF copy (DVE). K-bias is dropped: softmax over k is
    invariant to per-q constants, and bk only contributes bk.(Q+bq) which is
    constant in k. Q-bias is folded into the projection via a rank-1
    (bq-row x ones) matmul into the same PSUM accumulation.
  - V' = [ones | V_h] natural layout [S, 97*4] via augmented weight matrix
    (bias + ones column folded into the projection contraction); ones column
    FIRST so the softmax denominator lands on PSUM partition 0.
  - scoresT[k, q] = KT_h^T @ QT_h per 128-row k-tile and 512-col q-chunk in
    bf16 (1 cycle/col at any width), computing only cols >= rt for diagonal
    tiles; exp on ScalarE (PSUM->SBUF bf16, scale=1/sqrt(Hd) folded in);
    causal diagonal blocks zeroed post-exp by a 0/1 mask multiply on DVE
  - OT' accumulated in PSUM: row 0 = softmax denominator, rows 1..96 =
    unnormalized head output (d x q)
  - reciprocal (custom DVE approx, reads PSUM row 0 directly) ->
    partition_broadcast (Pool) -> normalize rows 0..96 on DVE; row 0 becomes
    den * (1/den) = 1.0, which doubles as the ones-row that folds the output
    bias into head 0's output-projection matmul (97-partition contraction
    against wo rows [bo ; Wo_h]).
  - output projection summed in PSUM across heads, copy to SBUF on DVE,
    DMA to HBM
"""

import os
import sys

sys.path.insert(0, "/opt/trn_rl_repo")

import numpy as np

import concourse.bass as bass
import concourse.tile as tile
from concourse import bacc, mybir
from concourse.bass_utils import run_bass_kernel_spmd

N_CORES = 8
S = 2048
D = 384
H = 4
HD = 96
CH = 512          # q-chunk width (columns per matmul)
NCH = S // CH     # 4 q-chunks
P = 128           # k-tile height / partition dim
KTN = S // P      # 16 k-tiles
SCALE = 1.0 / np.sqrt(HD)

F32 = mybir.dt.float32
BF16 = mybir.dt.bfloat16
MM_DT = os.environ.get("ATTN_MM_DT", "float32r")  # float32r | float32


def build_nc(repeat=1, variant=(), loop_n=0):
    nc = bacc.Bacc("TRN2", target_bir_lowering=False, debug=False,
                   enable_asserts=False, num_devices=N_CORES)
    # MF: dtype for tensors feeding matmuls (float32r = single-pass relaxed
    # fp32 on the PE, 4x faster than true fp32; same 4-byte numpy layout).
    # AT: attention-core dtype (bf16: 1 cycle/col at any moving width).
    MF = mybir.dt.float32r if MM_DT == "float32r" else F32
    AT = BF16 if "at_f32" not in variant else MF

    xt_d = nc.dram_tensor("xt", [D, S], MF, kind="ExternalInput").ap()
    wq_d = nc.dram_tensor("wq", [D, D], MF, kind="ExternalInput").ap()
    wk_d = nc.dram_tensor("wk", [D, D], MF, kind="ExternalInput").ap()
    wvx_d = nc.dram_tensor("wvx", [D + 1, 97 * H], MF, kind="ExternalInput").ap()
    woa_d = nc.dram_tensor("woa", [97 * H, D], MF, kind="ExternalInput").ap()
    bqr_d = nc.dram_tensor("bqr", [1, D], MF, kind="ExternalInput").ap()
    msk_d = nc.dram_tensor("msk", [P, P], AT, kind="ExternalInput").ap()
    ones_d = nc.dram_tensor("onesrow", [1, S], MF, kind="ExternalInput").ap()
    out_d = nc.dram_tensor("out", [S, D], F32, kind="ExternalOutput").ap()

    Exp = mybir.ActivationFunctionType.Exp
    mult = mybir.AluOpType.mult

    with tile.TileContext(nc) as tc:
        wpool = tc.alloc_tile_pool(name="w", bufs=1)
        xpool = tc.alloc_tile_pool(name="x", bufs=1)
        qkt_pool = tc.alloc_tile_pool(name="qkt", bufs=1)
        vpool = tc.alloc_tile_pool(name="v", bufs=1)
        ppool = tc.alloc_tile_pool(name="p", bufs=4)
        onpool = tc.alloc_tile_pool(name="on", bufs=2)
        rpool = tc.alloc_tile_pool(name="r", bufs=3)
        GRP = 2
        qkpool = tc.alloc_tile_pool(name="qkps", bufs=2, space="PSUM")
        accpool = tc.alloc_tile_pool(name="accps", bufs=4, space="PSUM")

        import contextlib
        loop_ctx = (tc.For_i(0, loop_n, 1) if loop_n
                    else contextlib.nullcontext())
        with loop_ctx:
          for _rep in range(repeat):
              # ---- load weights / constants ----
              xt_sb, wq_sb, wk_sb, wv_sb, wo_sb = [], [], [], [], []
              for t in range(3):
                  xt = xpool.tile([P, S], MF, name=f"xt{t}", tag=f"xt{t}")
                  nc.sync.dma_start(xt[:], xt_d[P * t:P * t + P, :])
                  xt_sb.append(xt)
                  wqt = wpool.tile([P, D], MF, name=f"wq{t}", tag=f"wq{t}")
                  nc.sync.dma_start(wqt[:], wq_d[P * t:P * t + P, :])
                  wq_sb.append(wqt)
                  wkt = wpool.tile([P, D], MF, name=f"wk{t}", tag=f"wk{t}")
                  nc.sync.dma_start(wkt[:], wk_d[P * t:P * t + P, :])
                  wk_sb.append(wkt)
                  wvt = wpool.tile([P, 97 * H], MF, name=f"wv{t}", tag=f"wv{t}")
                  nc.sync.dma_start(wvt[:], wvx_d[P * t:P * t + P, :])
                  wv_sb.append(wvt)
              wvb = wpool.tile([1, 97 * H], MF, name="wvb", tag="wvb")
              nc.sync.dma_start(wvb[:], wvx_d[D:D + 1, :])
              for h in range(H):
                  wot = wpool.tile([97, D], MF, name=f"wo{h}", tag=f"wo{h}")
                  nc.sync.dma_start(wot[:], woa_d[97 * h:97 * h + 97, :])
                  wo_sb.append(wot)
              bqr_sb = wpool.tile([1, D], MF, name="bqr", tag="bqr")
              nc.sync.dma_start(bqr_sb[:], bqr_d[:, :])
              msk_sb = wpool.tile([P, P], AT, name="msk", tag="msk")
              nc.sync.dma_start(msk_sb[:], msk_d[:, :])
              ones = wpool.tile([1, S], MF, name="ones", tag="ones")
              nc.sync.dma_start(ones[:], ones_d[:, :])

              # ---- Q/K projections: per-head transposed layout [96, S] ----
              qt_sb, kt_sb = [], []
              for h in range(H):
                  qt = qkt_pool.tile([HD, S], AT, name=f"qt{h}", tag=f"qt{h}")
                  qt_sb.append(qt)
                  kt = qkt_pool.tile([HD, S], AT, name=f"kt{h}", tag=f"kt{h}")
                  kt_sb.append(kt)
              for w_sb, dst, has_bias in ((wq_sb, qt_sb, True),
                                          (wk_sb, kt_sb, False)):
                  for h in range(H):
                      for ci in range(NCH):
                          ps = accpool.tile([HD, CH], F32, name="projps", tag="acc")
                          for t in range(3):
                              nc.tensor.matmul(
                                  ps[:],
                                  w_sb[t][:, HD * h:HD * h + HD],
                                  xt_sb[t][:, CH * ci:CH * ci + CH],
                                  start=(t == 0), stop=(t == 2 and not has_bias))
                          if has_bias:
                              nc.tensor.matmul(
                                  ps[:], bqr_sb[:, HD * h:HD * h + HD],
                                  ones[:, 0:CH], start=False, stop=True)
                          nc.vector.tensor_copy(
                              dst[h][:, CH * ci:CH * ci + CH], ps[:])

              # ---- V' projection: natural layout [S, 97*H], ones col first --
              v_sb = []
              for st in range(KTN):
                  ps = accpool.tile([P, 97 * H], F32, name="vps", tag="acc")
                  for t in range(3):
                      nc.tensor.matmul(ps[:], xt_sb[t][:, P * st:P * st + P],
                                       wv_sb[t][:], start=(t == 0), stop=False)
                  nc.tensor.matmul(ps[:], ones[:, 0:P], wvb[:],
                                   start=False, stop=True)
                  vt = vpool.tile([P, 97 * H], AT, name=f"v{st}", tag=f"v{st}")
                  nc.vector.tensor_copy(vt[:], ps[:])
                  v_sb.append(vt)

              # ---- attention ----
              for ci in range(NCH):
                  on_tiles = []
                  for h in range(H):
                      nkt = 4 * (ci + 1)
                      acc = accpool.tile([P, CH], F32, name="acc", tag="acc")
                      for g in range(nkt // GRP):
                          kts = [GRP * g, GRP * g + 1]
                          qk = qkpool.tile([P, GRP * CH], F32, name="qk", tag="qk")
                          for j, kt in enumerate(kts):
                              rt = max(P * kt - CH * ci, 0)
                              nc.tensor.matmul(
                                  qk[:, CH * j + rt:CH * (j + 1)],
                                  kt_sb[h][:, P * kt:P * kt + P],
                                  qt_sb[h][:, CH * ci + rt:CH * ci + CH],
                                  start=True, stop=True)
                          pt = ppool.tile([P, GRP * CH], AT, name="pt", tag="pt")
                          # contiguous exp from the first tile's start col;
                          # cols of tile j+1 below its rt get exp(stale-PSUM)
                          # garbage but are never read by the PV matmul
                          rt0 = max(P * kts[0] - CH * ci, 0)
                          nc.scalar.activation(pt[:, rt0:GRP * CH],
                                               qk[:, rt0:GRP * CH],
                                               Exp, scale=float(SCALE))
                          for j, kt in enumerate(kts):
                              rt = P * kt - CH * ci
                              if rt >= 0:
                                  # zero the upper triangle of the 128x128
                                  # diagonal block; cols below rt are skipped
                                  # by the PV matmul
                                  nc.vector.tensor_mul(
                                      pt[:, CH * j + rt:CH * j + rt + P],
                                      pt[:, CH * j + rt:CH * j + rt + P],
                                      msk_sb[:, 0:P])
                          for j, kt in enumerate(kts):
                              rt = P * kt - CH * ci
                              scol = max(rt, 0)
                              nc.tensor.matmul(
                                  acc[0:97, scol:CH],
                                  v_sb[kt][:, 97 * h:97 * h + 97],
                                  pt[:, CH * j + scol:CH * (j + 1)],
                                  start=(kt == 0), stop=(kt == nkt - 1),
                                  skip_group_check=True)
                      # normalize: row 0 of acc is the softmax denominator
                      # (ones column of V' is first). reciprocal reads PSUM
                      # partition 0 directly; normalizing rows 0..96 makes
                      # row 0 = den*(1/den) = 1.0, the ones-row consumed by
                      # the bias row of the output projection.
                      den0 = rpool.tile([1, CH], F32, name="den0", tag="den0")
                      nc.vector.reciprocal_approx_fast(out=den0[:], in_=acc[0:1, :])
                      rb = rpool.tile([97, CH], F32, name="rb", tag="rb")
                      nc.gpsimd.partition_broadcast(rb[:], den0[:], channels=97)
                      on = onpool.tile([97, CH], MF, name=f"on{h}", tag=f"on{h}")
                      nc.vector.tensor_tensor(on[:], acc[0:97, :], rb[:], op=mult)
                      on_tiles.append(on)
                  # output projection for this chunk's 4 row-tiles; head 0's
                  # wo rows are [bo ; Wo_0] against on row 0 == 1.0 (bias),
                  # heads 1..3 have a zero row there.
                  for sj in range(4):
                      st = 4 * ci + sj
                      fo = accpool.tile([P, D], F32, name="fo", tag="acc")
                      for h in range(H):
                          nc.tensor.matmul(fo[:], on_tiles[h][:, P * sj:P * sj + P],
                                           wo_sb[h][:], start=(h == 0),
                                           stop=(h == H - 1))
                      fs = onpool.tile([P, D], F32, name="fs", tag="fs", bufs=3)
                      nc.vector.tensor_copy(fs[:], fo[:])
                      nc.sync.dma_start(out_d[P * st:P * st + P, :], fs[:])

        for pool in (accpool, qkpool, rpool, onpool, ppool, vpool,
                     qkt_pool, xpool, wpool):
            pool.release()

    nc.finalize()
    return nc


_NC_CACHE = None


def get_nc():
    global _NC_CACHE
    if _NC_CACHE is None:
        _NC_CACHE = build_nc()
    return _NC_CACHE


def host_prep(x, Wq, bq, Wk, bk, Wv, bv, Wo, bo):
    """Build per-core input maps (layout prep only; all FLOPs run on device)."""
    x = np.ascontiguousarray(np.asarray(x, dtype=np.float32))
    Wq = np.ascontiguousarray(np.asarray(Wq, dtype=np.float32))
    Wk = np.ascontiguousarray(np.asarray(Wk, dtype=np.float32))
    Wv = np.ascontiguousarray(np.asarray(Wv, dtype=np.float32))
    Wo = np.ascontiguousarray(np.asarray(Wo, dtype=np.float32))
    bq = np.asarray(bq, dtype=np.float32)
    bv = np.asarray(bv, dtype=np.float32)
    bo = np.asarray(bo, dtype=np.float32)
    # bk is dropped: scores = K^T(Q+bq) + (bk . (Q+bq))(q), and the second
    # term is constant over k at fixed q, so softmax ignores it.

    # V' weights: ones column first, then the 96 head dims
    wvx = np.zeros((D + 1, 97 * H), np.float32)
    for h in range(H):
        wvx[:D, 97 * h + 1:97 * h + 97] = Wv[:, HD * h:HD * h + HD]
        wvx[D, 97 * h + 1:97 * h + 97] = bv[HD * h:HD * h + HD]
        wvx[D, 97 * h] = 1.0

    # output projection rows per head: [bias-or-zero row ; Wo_h]
    woa = np.zeros((97 * H, D), np.float32)
    for h in range(H):
        woa[97 * h + 1:97 * h + 97, :] = Wo[HD * h:HD * h + HD, :]
    woa[0, :] = bo

    # upper-triangle zero mask for the 128x128 diagonal block
    import ml_dtypes
    jj = np.arange(P)[None, :]
    pp = np.arange(P)[:, None]
    msk16 = (jj >= pp).astype(ml_dtypes.bfloat16)

    bqr = np.ascontiguousarray(bq.reshape(1, D))
    common = dict(wq=Wq, wk=Wk, wvx=wvx, woa=woa, bqr=bqr, msk=msk16,
                  onesrow=np.ones((1, S), np.float32))
    return [dict(xt=np.ascontiguousarray(x[b].T), **common)
            for b in range(x.shape[0])]


def kernel(**inputs):
    in_maps = host_prep(**inputs)
    nc = get_nc()
    res = run_bass_kernel_spmd(nc, in_maps, core_ids=list(range(N_CORES)))
    return np.stack([res.results[b]["out"] for b in range(N_CORES)], axis=0)
